# revision 5
# baseline (speedup 1.0000x reference)
"""Multihead attention (B=2, L=2048, D=1024, 16 heads) on 8 trn2 cores.

Sharding: tensor-parallel over heads — 2 heads per core. Each core computes
q/k/v projections for its 128 columns of Wq/Wk/Wv, full attention for its two
heads, and a partial output projection against its 128 rows of Wo. The host
sums the 8 partials and adds bo.

Per-core layouts (all built from a host-side transpose of x):
  qT/kT: [128(d_local), B*L]   — contraction-major for the scoresT matmuls
  v:     [s, 64]+ones column   — lhsT for attn@v; the ones column makes the
                                 PE emit the softmax denominator as row 64
  scoresT[s, l] per (b, l-chunk), exp'd on ScalarE (no max subtraction:
  scores ~ N(0,1) for this model, exp is far from overflow), attn@v
  accumulated over s-tiles in PSUM, normalized by a PE-broadcast reciprocal.
"""

from contextlib import ExitStack

import numpy as np

import concourse.bacc as bacc
import concourse.mybir as mybir
import concourse.tile as tile
from concourse.bass_utils import run_bass_kernel_spmd

D_MODEL = 1024
N_HEAD = 16
HEAD_DIM = 64
B = 2
L = 2048
N_CORES = 8
HPC = N_HEAD // N_CORES  # heads per core
MLOC = HPC * HEAD_DIM  # 128: local d width per core

F32 = mybir.dt.float32
BF16 = mybir.dt.bfloat16


def build_nc(Lb=L, lc_size=512, nch=512):
    """Build the per-core Bass program. Lb = sequence length per batch."""
    BLb = B * Lb
    KT = D_MODEL // 128  # 8 contraction tiles for the projections
    n_nch = BLb // nch  # projection column chunks
    st_per_nch = nch // 128  # s-tiles per projection chunk
    n_lc = Lb // lc_size  # attention l-chunks per batch
    n_st = Lb // 128  # s-tiles per batch
    n_lt = lc_size // 128  # l-tiles (128) per l-chunk

    nc = bacc.Bacc("TRN2", target_bir_lowering=False, debug=False)

    xT = nc.dram_tensor("xT", [D_MODEL, BLb], F32, kind="ExternalInput").ap()
    wq = nc.dram_tensor("wq", [D_MODEL, MLOC], F32, kind="ExternalInput").ap()
    wk = nc.dram_tensor("wk", [D_MODEL, MLOC], F32, kind="ExternalInput").ap()
    wv = nc.dram_tensor("wv", [D_MODEL, MLOC], F32, kind="ExternalInput").ap()
    wo = nc.dram_tensor("wo", [MLOC, D_MODEL], F32, kind="ExternalInput").ap()
    bq = nc.dram_tensor("bq", [MLOC, 1], F32, kind="ExternalInput").ap()
    bk = nc.dram_tensor("bk", [MLOC, 1], F32, kind="ExternalInput").ap()
    bv = nc.dram_tensor("bv", [MLOC, 1], F32, kind="ExternalInput").ap()
    out = nc.dram_tensor("out", [BLb, D_MODEL], F32, kind="ExternalOutput").ap()

    with tile.TileContext(nc) as tc, ExitStack() as ctx:
        consts = ctx.enter_context(tc.tile_pool(name="consts", bufs=1))
        qk_sb = ctx.enter_context(tc.tile_pool(name="qk_sb", bufs=1))

        # Weights resident in SBUF: [128, KT*128], k-tile k at cols [128k, 128k+128).
        wq_sb = consts.tile([128, KT, MLOC], F32, tag="wq")
        wk_sb = consts.tile([128, KT, MLOC], F32, tag="wk")
        wv_sb = consts.tile([128, KT, MLOC], F32, tag="wv")
        wo_sb = consts.tile([128, D_MODEL], F32, tag="wo")
        for w_sb, w_dram in ((wq_sb, wq), (wk_sb, wk), (wv_sb, wv)):
            nc.sync.dma_start(w_sb[:], w_dram.rearrange("(k p) m -> p k m", p=128))
        nc.sync.dma_start(wo_sb[:], wo)
        bq_sb = consts.tile([MLOC, 1], F32, tag="bq")
        bk_sb = consts.tile([MLOC, 1], F32, tag="bk")
        bv_sb = consts.tile([MLOC, 1], F32, tag="bv")
        for b_sb, b_dram in ((bq_sb, bq), (bk_sb, bk), (bv_sb, bv)):
            nc.sync.dma_start(b_sb[:], b_dram)
        ones_sb = consts.tile([1, 128], F32, tag="ones")
        nc.vector.memset(ones_sb[:], 1.0)

        # Persistent activations.
        qT_sb = qk_sb.tile([128, BLb], F32, tag="qT")  # [d_local, b*Lb+l]
        kT_sb = qk_sb.tile([128, BLb], F32, tag="kT")
        # v (natural layout) + ones column, bf16: per (b, head): [128, n_st, 65]
        vaug = [
            [qk_sb.tile([128, n_st, HEAD_DIM + 1], BF16, tag=f"vaug{bi}{h}", name=f"vaug{bi}{h}")
             for h in range(HPC)]
            for bi in range(B)
        ]
        for bi in range(B):
            for h in range(HPC):
                nc.vector.memset(vaug[bi][h][:, :, HEAD_DIM:], 1.0)

        # ---- Phase 1: projections ----
        with (
            tc.tile_pool(name="xt", bufs=KT + 2) as xt_pool,
            tc.tile_pool(name="pj_ps", bufs=2, space="PSUM") as pj_ps,
            tc.tile_pool(name="vn_ps", bufs=2, space="PSUM") as vn_ps,
        ):
            for nc_i in range(n_nch):
                csl = slice(nc_i * nch, (nc_i + 1) * nch)
                xts = []
                for k in range(KT):
                    xt = xt_pool.tile([128, nch], F32, tag="xt")
                    nc.sync.dma_start(xt[:], xT[128 * k : 128 * (k + 1), csl])
                    xts.append(xt)
                ps_q = pj_ps.tile([128, nch], F32, tag="ps_q")
                ps_k = pj_ps.tile([128, nch], F32, tag="ps_k")
                for k in range(KT):
                    nc.tensor.matmul(ps_q[:], wq_sb[:, k, :], xts[k][:],
                                     start=(k == 0), stop=(k == KT - 1))
                    nc.tensor.matmul(ps_k[:], wk_sb[:, k, :], xts[k][:],
                                     start=(k == 0), stop=(k == KT - 1))
                nc.scalar.activation(qT_sb[:, csl], ps_q[:],
                                     mybir.ActivationFunctionType.Identity,
                                     bias=bq_sb[:], scale=1.0)
                nc.scalar.activation(kT_sb[:, csl], ps_k[:],
                                     mybir.ActivationFunctionType.Identity,
                                     bias=bk_sb[:], scale=1.0)
                # v in natural [s, d_local] layout: lhsT = xT k-tiles.
                ps_v = vn_ps.tile([128, nch], F32, tag="ps_v")
                for st in range(st_per_nch):
                    ssl = slice(128 * st, 128 * (st + 1))
                    for k in range(KT):
                        nc.tensor.matmul(ps_v[:, ssl], xts[k][:, ssl],
                                         wv_sb[:, k, :],
                                         start=(k == 0), stop=(k == KT - 1))
                for st in range(st_per_nch):
                    st_g = nc_i * st_per_nch + st
                    bi, st_b = divmod(st_g, n_st)
                    for h in range(HPC):
                        nc.vector.tensor_copy(
                            vaug[bi][h][:, st_b, :HEAD_DIM],
                            ps_v[:, 128 * st + HEAD_DIM * h
                                 : 128 * st + HEAD_DIM * (h + 1)])

        # ---- Phase 2: attention + output projection ----
        with (
            tc.tile_pool(name="sc_ps", bufs=1, space="PSUM") as sc_ps,
            tc.tile_pool(name="av_ps", bufs=1, space="PSUM") as av_ps,
            tc.tile_pool(name="r_ps", bufs=1, space="PSUM") as r_ps,
            tc.tile_pool(name="o_ps", bufs=2, space="PSUM") as o_ps,
            tc.tile_pool(name="expT", bufs=6) as exp_pool,
            tc.tile_pool(name="att_sb", bufs=2) as att_sb,
            tc.tile_pool(name="out_sb", bufs=3) as out_pool,
        ):
            for bi in range(B):
                for lc in range(n_lc):
                    lsl = slice(bi * Lb + lc * lc_size, bi * Lb + (lc + 1) * lc_size)
                    ps_av = [av_ps.tile([HEAD_DIM + 1, lc_size], F32, tag=f"av{h}", name=f"av{h}")
                             for h in range(HPC)]
                    for st in range(n_st):
                        ssl = slice(bi * Lb + st * 128, bi * Lb + (st + 1) * 128)
                        exps = []
                        for h in range(HPC):
                            hsl = slice(64 * h, 64 * (h + 1))
                            ps_sc = sc_ps.tile([128, lc_size], F32, tag=f"sc{h}")
                            nc.tensor.matmul(ps_sc[:], kT_sb[hsl, ssl],
                                             qT_sb[hsl, lsl],
                                             start=True, stop=True,
                                             tile_position=(64 * h, 0))
                            ex = exp_pool.tile([128, lc_size], BF16, tag=f"ex{h}")
                            nc.scalar.activation(
                                ex[:], ps_sc[:],
                                mybir.ActivationFunctionType.Exp,
                                scale=1.0 / np.sqrt(HEAD_DIM))
                            exps.append(ex)
                        for h in range(HPC):
                            nc.tensor.matmul(ps_av[h][:], vaug[bi][h][:, st, :],
                                             exps[h][:],
                                             start=(st == 0), stop=(st == n_st - 1))
                    # Normalize: denom is row 64 of each ps_av.
                    den = att_sb.tile([1, 2, lc_size], F32, tag="den")
                    rcp = att_sb.tile([128, 2, lc_size], F32, tag="rcp")
                    ps_r = r_ps.tile([128, 2, lc_size], F32, tag="ps_r")
                    for h in range(HPC):
                        nc.scalar.copy(den[0:1, h, :], ps_av[h][64:65, :])
                        nc.tensor.matmul(ps_r[:, h, :], ones_sb[:],
                                         den[0:1, h, :], start=True, stop=True)
                    nc.vector.reciprocal(rcp[:], ps_r[:])
                    oT = att_sb.tile([128, lc_size], F32, tag="oT")
                    for h in range(HPC):
                        hsl = slice(64 * h, 64 * (h + 1))
                        nc.vector.tensor_mul(oT[hsl, :], ps_av[h][:HEAD_DIM, :],
                                             rcp[hsl, h, :])
                        nc.vector.tensor_scalar_add(oT[hsl, :], oT[hsl, :],
                                                    bv_sb[hsl, :])
                    # Output projection for this l-chunk.
                    for lt in range(n_lt):
                        for dh in range(2):
                            ps_o = o_ps.tile([128, 512], F32, tag="ps_o")
                            nc.tensor.matmul(
                                ps_o[:], oT[:, 128 * lt : 128 * (lt + 1)],
                                wo_sb[:, 512 * dh : 512 * (dh + 1)],
                                start=True, stop=True)
                            ob = out_pool.tile([128, 512], F32, tag="ob")
                            nc.vector.tensor_copy(ob[:], ps_o[:])
                            nc.sync.dma_start(
                                out[bi * Lb + lc * lc_size + 128 * lt
                                    : bi * Lb + lc * lc_size + 128 * (lt + 1),
                                    512 * dh : 512 * (dh + 1)], ob[:])

    nc.compile()
    return nc


def make_in_maps(x, Wq, bq, Wk, bk, Wv, bv, Wo, Lb=L):
    """Per-core input dicts from full inputs."""
    BLb = B * Lb
    xT = np.ascontiguousarray(x.reshape(BLb, D_MODEL).T)
    in_maps = []
    for c in range(N_CORES):
        dsl = slice(MLOC * c, MLOC * (c + 1))
        in_maps.append({
            "xT": xT,
            "wq": np.ascontiguousarray(Wq[:, dsl]),
            "wk": np.ascontiguousarray(Wk[:, dsl]),
            "wv": np.ascontiguousarray(Wv[:, dsl]),
            "wo": np.ascontiguousarray(Wo[dsl, :]),
            "bq": np.ascontiguousarray(bq[dsl].reshape(MLOC, 1)),
            "bk": np.ascontiguousarray(bk[dsl].reshape(MLOC, 1)),
            "bv": np.ascontiguousarray(bv[dsl].reshape(MLOC, 1)),
        })
    return in_maps


_NC_CACHE = {}


def _get_nc():
    if "nc" not in _NC_CACHE:
        _NC_CACHE["nc"] = build_nc()
    return _NC_CACHE["nc"]


def kernel(x, Wq, bq, Wk, bk, Wv, bv, Wo, bo):
    x = np.asarray(x, dtype=np.float32)
    nc = _get_nc()
    in_maps = make_in_maps(np.asarray(x), np.asarray(Wq), np.asarray(bq),
                           np.asarray(Wk), np.asarray(bk), np.asarray(Wv),
                           np.asarray(bv), np.asarray(Wo))
    res = run_bass_kernel_spmd(nc, in_maps, list(range(N_CORES)))
    acc = np.zeros((B * L, D_MODEL), dtype=np.float32)
    for c in range(N_CORES):
        acc += res.results[c]["out"]
    acc += np.asarray(bo, dtype=np.float32)
    return acc.reshape(B, L, D_MODEL)


# revision 6
# speedup vs baseline: 2.0687x; 2.0687x over previous
"""Multihead attention (B=2, L=2048, D=1024, 16 heads) on 8 trn2 cores.

Sharding: tensor-parallel over heads — 2 heads per core. Each core computes
q/k/v projections for its 128 columns of Wq/Wk/Wv, full attention for its two
heads, and a partial output projection against its 128 rows of Wo. The host
sums the 8 partials and adds bo.

Compute is bf16 on the PE (fp32 matmuls run as two LOW_HIGH passes and get no
fast-weight-load; bf16 halves PE work and quarters LDWEIGHTS cost), with fp32
PSUM accumulation everywhere.

Per-core layouts (all built from a host-side transpose+cast of x):
  qT/kT: [128(d_local), B*L]   — contraction-major for the scoresT matmuls
  v:     [s, 64]+ones column   — lhsT for attn@v; the ones column makes the
                                 PE emit the softmax denominator as row 64
  scoresT[s, l] per (b, l-chunk), both heads in one 2-bank PSUM tile so one
  ScalarE exp covers them (no max subtraction: scores ~ N(0,1) for this
  model, exp is far from overflow), attn@v accumulated over s-tiles in PSUM,
  normalized by a PE-broadcast fast reciprocal.
"""

from contextlib import ExitStack

import ml_dtypes
import numpy as np

import concourse.bacc as bacc
import concourse.mybir as mybir
import concourse.tile as tile
from concourse.bass_utils import run_bass_kernel_spmd

D_MODEL = 1024
N_HEAD = 16
HEAD_DIM = 64
B = 2
L = 2048
N_CORES = 8
HPC = N_HEAD // N_CORES  # heads per core
MLOC = HPC * HEAD_DIM  # 128: local d width per core

F32 = mybir.dt.float32
BF16 = mybir.dt.bfloat16
NPBF16 = ml_dtypes.bfloat16


def build_nc(Lb=L, lc_size=512, nch=512):
    """Build the per-core Bass program. Lb = sequence length per batch."""
    BLb = B * Lb
    KT = D_MODEL // 128  # 8 contraction tiles for the projections
    n_nch = BLb // nch  # projection column chunks
    st_per_nch = nch // 128  # s-tiles per projection chunk
    n_lc = Lb // lc_size  # attention l-chunks per batch
    n_st = Lb // 128  # s-tiles per batch
    n_lt = lc_size // 128  # l-tiles (128) per l-chunk

    nc = bacc.Bacc("TRN2", target_bir_lowering=False, debug=False)

    xT = nc.dram_tensor("xT", [D_MODEL, BLb], BF16, kind="ExternalInput").ap()
    wq = nc.dram_tensor("wq", [D_MODEL, MLOC], BF16, kind="ExternalInput").ap()
    wk = nc.dram_tensor("wk", [D_MODEL, MLOC], BF16, kind="ExternalInput").ap()
    wv = nc.dram_tensor("wv", [D_MODEL, MLOC], BF16, kind="ExternalInput").ap()
    wo = nc.dram_tensor("wo", [MLOC, D_MODEL], BF16, kind="ExternalInput").ap()
    bq = nc.dram_tensor("bq", [MLOC, 1], F32, kind="ExternalInput").ap()
    bk = nc.dram_tensor("bk", [MLOC, 1], F32, kind="ExternalInput").ap()
    bv = nc.dram_tensor("bv", [MLOC, 1], F32, kind="ExternalInput").ap()
    out = nc.dram_tensor("out", [BLb, D_MODEL], F32, kind="ExternalOutput").ap()

    with tile.TileContext(nc) as tc, ExitStack() as ctx:
        consts = ctx.enter_context(tc.tile_pool(name="consts", bufs=1))
        qk_sb = ctx.enter_context(tc.tile_pool(name="qk_sb", bufs=1))

        # Weights resident in SBUF: k-tile k of w* at [:, k, :].
        wq_sb = consts.tile([128, KT, MLOC], BF16, tag="wq")
        wk_sb = consts.tile([128, KT, MLOC], BF16, tag="wk")
        wv_sb = consts.tile([128, KT, MLOC], BF16, tag="wv")
        wo_sb = consts.tile([128, D_MODEL], BF16, tag="wo")
        for w_sb, w_dram in ((wq_sb, wq), (wk_sb, wk), (wv_sb, wv)):
            nc.sync.dma_start(w_sb[:], w_dram.rearrange("(k p) m -> p k m", p=128))
        nc.sync.dma_start(wo_sb[:], wo)
        bq_sb = consts.tile([MLOC, 1], F32, tag="bq")
        bk_sb = consts.tile([MLOC, 1], F32, tag="bk")
        bv_sb = consts.tile([MLOC, 1], F32, tag="bv")
        for b_sb, b_dram in ((bq_sb, bq), (bk_sb, bk), (bv_sb, bv)):
            nc.sync.dma_start(b_sb[:], b_dram)
        ones_sb = consts.tile([1, 128], BF16, tag="ones")
        nc.vector.memset(ones_sb[:], 1.0)

        # Persistent activations.
        qT_sb = qk_sb.tile([128, BLb], BF16, tag="qT")  # [d_local, b*Lb+l]
        kT_sb = qk_sb.tile([128, BLb], BF16, tag="kT")
        # v (natural layout) + ones column: per (b, head): [128, n_st, 65]
        vaug = [
            [qk_sb.tile([128, n_st, HEAD_DIM + 1], BF16, tag=f"vaug{bi}{h}",
                        name=f"vaug{bi}{h}")
             for h in range(HPC)]
            for bi in range(B)
        ]
        for bi in range(B):
            for h in range(HPC):
                nc.vector.memset(vaug[bi][h][:, :, HEAD_DIM:], 1.0)

        # ---- Phase 1: projections ----
        with (
            tc.tile_pool(name="xt", bufs=KT + 2) as xt_pool,
            tc.tile_pool(name="pj_ps", bufs=2, space="PSUM") as pj_ps,
            tc.tile_pool(name="vn_ps", bufs=2, space="PSUM") as vn_ps,
        ):
            for nc_i in range(n_nch):
                csl = slice(nc_i * nch, (nc_i + 1) * nch)
                xts = []
                for k in range(KT):
                    xt = xt_pool.tile([128, nch], BF16, tag="xt")
                    nc.sync.dma_start(xt[:], xT[128 * k : 128 * (k + 1), csl])
                    xts.append(xt)
                ps_q = pj_ps.tile([128, nch], F32, tag="ps_q")
                ps_k = pj_ps.tile([128, nch], F32, tag="ps_k")
                for k in range(KT):
                    nc.tensor.matmul(ps_q[:], wq_sb[:, k, :], xts[k][:],
                                     start=(k == 0), stop=(k == KT - 1))
                    nc.tensor.matmul(ps_k[:], wk_sb[:, k, :], xts[k][:],
                                     start=(k == 0), stop=(k == KT - 1))
                nc.scalar.activation(qT_sb[:, csl], ps_q[:],
                                     mybir.ActivationFunctionType.Identity,
                                     bias=bq_sb[:], scale=1.0)
                nc.scalar.activation(kT_sb[:, csl], ps_k[:],
                                     mybir.ActivationFunctionType.Identity,
                                     bias=bk_sb[:], scale=1.0)
                # v in natural [s, d_local] layout: lhsT = xT k-tiles.
                ps_v = vn_ps.tile([128, nch], F32, tag="ps_v")
                for st in range(st_per_nch):
                    ssl = slice(128 * st, 128 * (st + 1))
                    for k in range(KT):
                        nc.tensor.matmul(ps_v[:, ssl], xts[k][:, ssl],
                                         wv_sb[:, k, :],
                                         start=(k == 0), stop=(k == KT - 1))
                for st in range(st_per_nch):
                    st_g = nc_i * st_per_nch + st
                    bi, st_b = divmod(st_g, n_st)
                    for h in range(HPC):
                        nc.vector.tensor_copy(
                            vaug[bi][h][:, st_b, :HEAD_DIM],
                            ps_v[:, 128 * st + HEAD_DIM * h
                                 : 128 * st + HEAD_DIM * (h + 1)])

        # ---- Phase 2: attention + output projection ----
        # PSUM: big pool (2-bank slots ×3) shared by scoresT / R-broadcast /
        # o-proj tiles; av pool 2 banks. Total 8 banks.
        with (
            tc.tile_pool(name="big_ps", bufs=3, space="PSUM") as big_ps,
            tc.tile_pool(name="av_ps", bufs=1, space="PSUM") as av_ps,
            tc.tile_pool(name="expT", bufs=4) as exp_pool,
            tc.tile_pool(name="att_sb", bufs=2) as att_sb,
            tc.tile_pool(name="out_sb", bufs=3) as out_pool,
        ):
            for bi in range(B):
                for lc in range(n_lc):
                    lsl = slice(bi * Lb + lc * lc_size, bi * Lb + (lc + 1) * lc_size)
                    ps_av = [av_ps.tile([HEAD_DIM + 1, lc_size], F32, tag=f"av{h}",
                                        name=f"av{h}")
                             for h in range(HPC)]
                    for st in range(n_st):
                        ssl = slice(bi * Lb + st * 128, bi * Lb + (st + 1) * 128)
                        ps_sc = big_ps.tile([128, HPC, lc_size], F32, tag="big",
                                            name="ps_sc")
                        for h in range(HPC):
                            hsl = slice(64 * h, 64 * (h + 1))
                            nc.tensor.matmul(ps_sc[:, h, :], kT_sb[hsl, ssl],
                                             qT_sb[hsl, lsl],
                                             start=True, stop=True,
                                             tile_position=(64 * h, 0))
                        ex = exp_pool.tile([128, HPC, lc_size], BF16, tag="ex")
                        nc.scalar.activation(ex[:], ps_sc[:],
                                             mybir.ActivationFunctionType.Exp,
                                             scale=1.0 / np.sqrt(HEAD_DIM))
                        for h in range(HPC):
                            nc.tensor.matmul(ps_av[h][:], vaug[bi][h][:, st, :],
                                             ex[:, h, :],
                                             start=(st == 0), stop=(st == n_st - 1))
                    # Normalize: denom is row 64 of each ps_av.
                    den = att_sb.tile([1, 2, lc_size], BF16, tag="den")
                    rcp = att_sb.tile([128, 2, lc_size], F32, tag="rcp")
                    ps_r = big_ps.tile([128, 2, lc_size], F32, tag="big", name="ps_r")
                    for h in range(HPC):
                        nc.scalar.copy(den[0:1, h, :], ps_av[h][64:65, :])
                        nc.tensor.matmul(ps_r[:, h, :], ones_sb[:],
                                         den[0:1, h, :], start=True, stop=True)
                    nc.vector.reciprocal_approx_fast(rcp[:], ps_r[:])
                    oT = att_sb.tile([128, lc_size], BF16, tag="oT")
                    for h in range(HPC):
                        hsl = slice(64 * h, 64 * (h + 1))
                        nc.vector.tensor_mul(oT[hsl, :], ps_av[h][:HEAD_DIM, :],
                                             rcp[hsl, h, :])
                        nc.vector.tensor_scalar_add(oT[hsl, :], oT[hsl, :],
                                                    bv_sb[hsl, :])
                    # Output projection for this l-chunk.
                    for lt in range(n_lt):
                        for dh in range(2):
                            ps_o = big_ps.tile([128, 512], F32, tag="big",
                                               name="ps_o")
                            nc.tensor.matmul(
                                ps_o[:], oT[:, 128 * lt : 128 * (lt + 1)],
                                wo_sb[:, 512 * dh : 512 * (dh + 1)],
                                start=True, stop=True)
                            ob = out_pool.tile([128, 512], F32, tag="ob")
                            nc.vector.tensor_copy(ob[:], ps_o[:])
                            nc.sync.dma_start(
                                out[bi * Lb + lc * lc_size + 128 * lt
                                    : bi * Lb + lc * lc_size + 128 * (lt + 1),
                                    512 * dh : 512 * (dh + 1)], ob[:])

    nc.compile()
    return nc


def make_in_maps(x, Wq, bq, Wk, bk, Wv, bv, Wo, Lb=L):
    """Per-core input dicts from full inputs."""
    BLb = B * Lb
    xT = np.ascontiguousarray(
        np.asarray(x, np.float32).reshape(BLb, D_MODEL).T).astype(NPBF16)
    Wq = np.asarray(Wq, np.float32).astype(NPBF16)
    Wk = np.asarray(Wk, np.float32).astype(NPBF16)
    Wv = np.asarray(Wv, np.float32).astype(NPBF16)
    Wo = np.asarray(Wo, np.float32).astype(NPBF16)
    in_maps = []
    for c in range(N_CORES):
        dsl = slice(MLOC * c, MLOC * (c + 1))
        in_maps.append({
            "xT": xT,
            "wq": np.ascontiguousarray(Wq[:, dsl]),
            "wk": np.ascontiguousarray(Wk[:, dsl]),
            "wv": np.ascontiguousarray(Wv[:, dsl]),
            "wo": np.ascontiguousarray(Wo[dsl, :]),
            "bq": np.ascontiguousarray(np.asarray(bq, np.float32)[dsl].reshape(MLOC, 1)),
            "bk": np.ascontiguousarray(np.asarray(bk, np.float32)[dsl].reshape(MLOC, 1)),
            "bv": np.ascontiguousarray(np.asarray(bv, np.float32)[dsl].reshape(MLOC, 1)),
        })
    return in_maps


_NC_CACHE = {}


def _get_nc():
    if "nc" not in _NC_CACHE:
        _NC_CACHE["nc"] = build_nc()
    return _NC_CACHE["nc"]


def kernel(x, Wq, bq, Wk, bk, Wv, bv, Wo, bo):
    nc = _get_nc()
    in_maps = make_in_maps(x, Wq, bq, Wk, bk, Wv, bv, Wo)
    res = run_bass_kernel_spmd(nc, in_maps, list(range(N_CORES)))
    acc = np.zeros((B * L, D_MODEL), dtype=np.float32)
    for c in range(N_CORES):
        acc += res.results[c]["out"]
    acc += np.asarray(bo, dtype=np.float32)
    return acc.reshape(B, L, D_MODEL)
